# revision 1
# baseline (speedup 1.0000x reference)
"""AutoCorrelation Trainium2 kernel.

Reference reformulation (verified to 3e-7 rel):
  H=8, L=2048, D=512, k_sel=4, SCALE=1/(H*L)
  qbar = sum_l queries[b,l,:];  qs = qbar @ wq;  t = wk @ qs
  mean_corr = (keys[b] @ t) * SCALE                     # [2048]
  top_idx, top_vals = top_k(mean_corr, 4); w = softmax(top_vals)
  Vp = values[b] @ wv                                   # [2048, 512]
  Aw = sum_j w_j * roll(Vp, -top_idx_j, axis=0)         # [2048, 512]
  # reference's transpose(0,3,1,2).reshape quirk => per output row i:
  #   r = i%4, c = ((i%32)//4)*64 + i//32
  #   out[b,i,:] = Aw[r*512:(r+1)*512, c] @ wo
  => for each r: out_rows(r) = Aw[r*512:(r+1)*512, :].T @ wo

Sharding: 8 cores = 4 batches x 2 channel-halves (d half of each head).
Each core redundantly computes the tiny front-end (top-k weights) for its
batch and produces the 1024 output rows whose channels fall in its half.

Device program per core:
  - qbar via PE (ones^T @ q-tiles, PSUM accumulate), qs via PE
  - t = rowsum(wk * bcast(qs)), mc = rowsum(keys * bcast(t)) on DVE
  - top-8 via DVE max/max_index on mc flattened to [1,2048] (DRAM bounce)
  - softmax on top-4 (ACT exp), w_j * I identities on DVE
  - VpT[c_half, l] = wv_half^T @ values^T  (PE, from host-transposed vt)
  - AwT[c, l-block] accumulated over j in PSUM via scaled-identity matmuls
    with register-offset slices (roll == dynamic free-dim slice on doubled VpT)
  - PE-transpose AwT -> Aw tiles, final GEMM Aw_r^T-tiles @ wo
  All heavy matmuls in float32r (full PE rate, ~1e-7 rel precision).
"""

import numpy as np

B, L, D = 4, 2048, 512
H = 8
K_SEL = 4
SCALE = 1.0 / (H * L)
N_CORES = 8
P = 128
CH = 256          # channels per core (half of 512)
NT = L // P       # 16 l-tiles
DK = D // P       # 4 d-tiles


def _build_nc():
    import concourse.bass as bass
    import concourse.bacc as bacc
    import concourse.mybir as mybir
    from concourse.tile import TileContext
    from concourse.masks import make_identity

    fp32 = mybir.dt.float32
    f32r = mybir.dt.float32r
    u32 = mybir.dt.uint32
    i32 = mybir.dt.int32
    AX = mybir.AxisListType.X
    MUL = mybir.AluOpType.mult

    nc = bacc.Bacc("TRN2", target_bir_lowering=False, debug=False, num_devices=N_CORES)

    q_dram = nc.dram_tensor("q", [L, D], f32r, kind="ExternalInput")
    kt_dram = nc.dram_tensor("kt", [D, L], f32r, kind="ExternalInput")   # keys^T
    vt_dram = nc.dram_tensor("vt", [D, L], f32r, kind="ExternalInput")   # values^T
    wq_dram = nc.dram_tensor("wq", [D, D], fp32, kind="ExternalInput")
    wkt_dram = nc.dram_tensor("wkt", [D, D], fp32, kind="ExternalInput")  # wk^T
    wvh_dram = nc.dram_tensor("wvh", [D, CH], f32r, kind="ExternalInput")
    wo_dram = nc.dram_tensor("wo", [D, D], f32r, kind="ExternalInput")
    ones_dram = nc.dram_tensor("ones", [P, 1], f32r, kind="ExternalInput")
    onesr_dram = nc.dram_tensor("onesr", [1, P], fp32, kind="ExternalInput")
    out_dram = nc.dram_tensor("out", [L // 2, D], fp32, kind="ExternalOutput")

    with TileContext(nc) as tc:
        with (
            tc.tile_pool(name="const", bufs=1) as cpool,
            tc.tile_pool(name="wts", bufs=1) as wts,
            tc.tile_pool(name="big", bufs=1) as big,
            tc.tile_pool(name="stream", bufs=2) as stream,
            tc.tile_pool(name="small", bufs=1) as small,
            tc.tile_pool(name="ps_fe", bufs=2, space="PSUM") as ps_fe,
            tc.tile_pool(name="ps_mm", bufs=3, space="PSUM") as ps_mm,
            tc.tile_pool(name="ps_tp", bufs=2, space="PSUM") as ps_tp,
        ):
            ident = cpool.tile([P, P], fp32, tag="ident")
            make_identity(nc, ident)
            identr = cpool.tile([P, P], f32r, tag="identr")
            nc.scalar.copy(identr, ident)
            ones_col = cpool.tile([P, 1], f32r, tag="ones")
            nc.sync.dma_start(ones_col, ones_dram[:, :])
            ones_row = cpool.tile([1, P], fp32, tag="onesr")
            nc.sync.dma_start(ones_row, onesr_dram[:, :])

            wq_sb = [wts.tile([P, D], fp32, tag=f"wq{i}", name=f"wq{i}") for i in range(DK)]
            wkt_sb = [wts.tile([P, D], fp32, tag=f"wkt{i}", name=f"wkt{i}") for i in range(DK)]
            wvh_sb = [wts.tile([P, CH], f32r, tag=f"wvh{i}", name=f"wvh{i}") for i in range(DK)]
            wo_sb = [wts.tile([P, D], f32r, tag=f"wo{i}", name=f"wo{i}") for i in range(DK)]
            kt_sb = [big.tile([P, L], f32r, tag=f"kt{i}", name=f"kt{i}") for i in range(DK)]
            vt_sb = [big.tile([P, L], f32r, tag=f"vt{i}", name=f"vt{i}") for i in range(DK)]

            # ---- q on two DMA queues; qbar row accumulate on PE ----
            ps_qbar = ps_fe.tile([1, D], fp32, tag="fe")
            qv = q_dram.rearrange("(g n p) d -> g p n d", p=P, n=2)
            for g in range(8):
                qt = stream.tile([P, 2, D], f32r, tag=f"qtile{g % 2}",
                                 name=f"qt{g}", bufs=1)
                (nc.sync if g % 2 == 0 else nc.scalar).dma_start(qt, qv[g])
                for n in range(2):
                    nc.tensor.matmul(
                        ps_qbar, ones_col, qt[:, n],
                        start=(g == 0 and n == 0), stop=(g == 7 and n == 1),
                    )
            # wire: sync gets wq + wvh + vt; vector gets wkt + kt + wo
            for i in range(DK):
                nc.sync.dma_start(wq_sb[i], wq_dram[i * P:(i + 1) * P, :])
            for i in range(DK):
                nc.scalar.dma_start(wkt_sb[i], wkt_dram[i * P:(i + 1) * P, :])
            for i in range(DK):
                nc.sync.dma_start(wvh_sb[i], wvh_dram[i * P:(i + 1) * P, :])
            for i in range(DK):
                nc.scalar.dma_start(kt_sb[i], kt_dram[i * P:(i + 1) * P, :])
            for i in range(DK):
                nc.sync.dma_start(vt_sb[i], vt_dram[i * P:(i + 1) * P, :])
            for i in range(DK):
                nc.scalar.dma_start(wo_sb[i], wo_dram[i * P:(i + 1) * P, :])

            qbar_row = small.tile([1, D], fp32, tag="qbar_row")
            nc.scalar.copy(qbar_row, ps_qbar)
            qbcol = small.tile([P, DK], fp32, tag="qbcol")
            for kk in range(DK):
                ptq = ps_tp.tile([P, 1], fp32, tag="tp")
                nc.tensor.transpose(
                    ptq, qbar_row[0:1, kk * P:(kk + 1) * P], ident[0:1, 0:1])
                nc.scalar.copy(qbcol[:, kk:kk + 1], ptq)

            # ---- u = qs^T = wq^T @ qbar^T as column chunks [128, 4] ----
            u_sb = small.tile([P, DK], fp32, tag="u_sb")
            for m in range(DK):
                psu = ps_tp.tile([P, 1], fp32, tag="tp")
                for kk in range(DK):
                    nc.tensor.matmul(
                        psu, wq_sb[kk][:, m * P:(m + 1) * P], qbcol[:, kk:kk + 1],
                        start=(kk == 0), stop=(kk == DK - 1),
                    )
                nc.scalar.copy(u_sb[:, m:m + 1], psu)

            # ---- t = wk @ u as column chunks (wk^T tiles as lhsT) ----
            t_sb = small.tile([P, DK], f32r, tag="t_sb")
            for ic in range(DK):
                pst = ps_tp.tile([P, 1], fp32, tag="tp")
                for mk in range(DK):
                    nc.tensor.matmul(
                        pst, wkt_sb[mk][:, ic * P:(ic + 1) * P], u_sb[:, mk:mk + 1],
                        start=(mk == 0), stop=(mk == DK - 1),
                    )
                nc.scalar.copy(t_sb[:, ic:ic + 1], pst)

            # ---- mean_corr directly as [1, 2048] on PE: t^T @ keys^T ----
            mc_flat = small.tile([1, L], fp32, tag="mc_flat")
            for nch in range(4):
                psm = ps_fe.tile([1, 512], fp32, tag="fe")
                for dk in range(DK):
                    nc.tensor.matmul(
                        psm, t_sb[:, dk:dk + 1],
                        kt_sb[dk][:, nch * 512:(nch + 1) * 512],
                        start=(dk == 0), stop=(dk == DK - 1),
                    )
                nc.scalar.copy(mc_flat[0:1, nch * 512:(nch + 1) * 512], psm)

            # ---- top-8 ----
            mx8 = small.tile([1, 8], fp32, tag="mx8")
            mi8 = small.tile([1, 8], u32, tag="mi8")
            nc.vector.max(out=mx8, in_=mc_flat)
            nc.vector.max_index(out=mi8, in_max=mx8, in_values=mc_flat)

            # ---- softmax over top-4 ----
            e4 = small.tile([1, K_SEL], fp32, tag="e4")
            nc.scalar.activation(
                e4, mx8[0:1, 0:K_SEL], mybir.ActivationFunctionType.Exp,
                scale=float(SCALE),
            )
            s1 = small.tile([1, 1], fp32, tag="s1")
            nc.vector.reduce_sum(s1, e4, axis=AX)
            r1 = small.tile([1, 1], fp32, tag="r1")
            nc.vector.reciprocal(r1, s1)
            w4 = small.tile([1, K_SEL], fp32, tag="w4")
            nc.vector.tensor_scalar(w4, e4, r1[0:1, 0:1], None, op0=MUL)

            # broadcast w4 to [128, 4] via PE: ones_row.T @ w4
            ps_wb = ps_fe.tile([P, K_SEL], fp32, tag="fe")
            nc.tensor.matmul(ps_wb, ones_row, w4, start=True, stop=True)
            wb = small.tile([P, K_SEL], fp32, tag="wb_sb")
            nc.scalar.copy(wb, ps_wb)
            wjI = [small.tile([P, P], f32r, tag=f"wjI{j}", name=f"wjI{j}")
                   for j in range(K_SEL)]
            for j in range(K_SEL):
                nc.vector.tensor_scalar(wjI[j], ident, wb[:, j:j + 1], None, op0=MUL)

            # ---- VpT = wv_half^T @ values^T, doubled along l ----
            vpT = big.tile([P, 2 * 2 * L], f32r, tag="vpT", name="vpT")
            for ct in range(2):
                for lc in range(4):
                    pv = ps_mm.tile([P, 512], fp32, tag="mm")
                    for dk in range(DK):
                        nc.tensor.matmul(
                            pv,
                            wvh_sb[dk][:, ct * P:(ct + 1) * P],
                            vt_sb[dk][:, lc * 512:(lc + 1) * 512],
                            start=(dk == 0), stop=(dk == DK - 1),
                        )
                    o = ct * 2 * L + lc * 512
                    nc.scalar.copy(vpT[:, o:o + 512], pv)
                nc.scalar.copy(
                    vpT[:, ct * 2 * L + L: ct * 2 * L + 2 * L],
                    vpT[:, ct * 2 * L: ct * 2 * L + L])

            # ---- per-j plain dynamic-slice copies (4 dynamic APs) ----
            vp3 = vpT.rearrange("p (c x) -> p c x", c=2)
            ws = [big.tile([P, 2, L], f32r, tag=f"vt{j}", name=f"ws{j}")
                  for j in range(K_SEL)]
            for j in range(K_SEL):
                eng = mybir.EngineType.Activation if j % 2 == 0 else mybir.EngineType.DVE
                s_j = nc.values_load(
                    mi8[0:1, j:j + 1].bitcast(i32),
                    engines=(eng,),
                    min_val=0, max_val=L - 1,
                    skip_runtime_bounds_check=True,
                )
                dyn = vp3[:, :, bass.ds(s_j, L)]
                if j % 2 == 0:
                    nc.scalar.copy(ws[j], dyn)
                else:
                    nc.vector.tensor_copy(ws[j], dyn)

            # ---- AwT = sum_j w_j ws_j via PSUM identity-matmul accumulation,
            # then transpose, per (ct, r) region ----
            aw = [[small.tile([P, CH], f32r, tag=f"aw_{lp}", bufs=2, name=f"aw{r}_{lp}") for lp in range(4)]
                  for r in range(4)]
            for r in range(4):
                for ct in range(2):
                    pa = ps_mm.tile([P, 512], fp32, tag="mm")
                    for j in range(K_SEL):
                        nc.tensor.matmul(
                            pa, wjI[j],
                            ws[j][:, ct, r * 512:(r + 1) * 512],
                            start=(j == 0), stop=(j == K_SEL - 1),
                        )
                    awT = small.tile([P, 512], f32r, tag="awT", bufs=2, name=f"awT{r}_{ct}")
                    nc.scalar.copy(awT, pa)
                    for lp in range(4):
                        pt = ps_tp.tile([P, P], f32r, tag="tp")
                        nc.tensor.transpose(pt, awT[:, lp * P:(lp + 1) * P], identr)
                        if (ct * 4 + lp) % 2 == 0:
                            nc.scalar.copy(aw[r][lp][:, ct * P:(ct + 1) * P], pt)
                        else:
                            nc.vector.tensor_copy(aw[r][lp][:, ct * P:(ct + 1) * P], pt)

            # ---- out rows ----
            for r in range(4):
                for cm in range(2):
                    po = ps_mm.tile([P, D], fp32, tag="mm")
                    for lp in range(4):
                        nc.tensor.matmul(
                            po,
                            aw[r][lp][:, cm * P:(cm + 1) * P],
                            wo_sb[lp],
                            start=(lp == 0), stop=(lp == DK - 1),
                        )
                    ot = stream.tile([P, D], fp32, tag="otile")
                    nc.scalar.copy(ot, po)
                    row0 = r * 256 + cm * P
                    nc.sync.dma_start(out_dram[row0:row0 + P, :], ot)

    nc.compile()
    return nc


_NC_CACHE = None


def _get_nc():
    global _NC_CACHE
    if _NC_CACHE is None:
        _NC_CACHE = _build_nc()
    return _NC_CACHE


def _half_cols(half):
    d0 = 32 * half
    return np.array([(cl // 32) * 64 + d0 + cl % 32 for cl in range(CH)])


def _row_index(half):
    # device row r*256 + cl  ->  full-output row i
    d0 = 32 * half
    idx = np.empty(1024, np.int64)
    for r in range(4):
        for cl in range(CH):
            i = (d0 + cl % 32) * 32 + (cl // 32) * 4 + r
            idx[r * CH + cl] = i
    return idx


def make_in_maps(queries, keys, values, wq, wk, wv, wo):
    ones = np.ones((P, 1), np.float32)
    in_maps = []
    for c in range(N_CORES):
        b, half = c // 2, c % 2
        vt = np.ascontiguousarray(values[b].T)  # [512, 2048]
        kt = np.ascontiguousarray(keys[b].T)    # [512, 2048]
        wvh = np.ascontiguousarray(wv[:, _half_cols(half)])
        wkt = np.ascontiguousarray(wk.T)
        in_maps.append({
            "q": np.ascontiguousarray(queries[b]),
            "kt": kt,
            "vt": vt,
            "wq": wq, "wkt": wkt, "wvh": wvh, "wo": wo,
            "ones": ones, "onesr": np.ones((1, P), np.float32),
        })
    return in_maps


def kernel(queries, keys, values, wq, wk, wv, wo, trace=False):
    import sys
    if "/opt/trn_rl_repo" not in sys.path:
        sys.path.insert(0, "/opt/trn_rl_repo")
    from concourse import bass_utils

    nc = _get_nc()
    in_maps = make_in_maps(queries, keys, values, wq, wk, wv, wo)
    res = bass_utils.run_bass_kernel_spmd(
        nc, in_maps, core_ids=list(range(N_CORES)), trace=trace,
    )
    out = np.empty((B, L, D), np.float32)
    for c in range(N_CORES):
        b, half = c // 2, c % 2
        out[b, _row_index(half), :] = res.results[c]["out"]
    if trace:
        return out, res
    return out



# revision 7
# speedup vs baseline: 1.8619x; 1.8619x over previous
"""AutoCorrelation Trainium2 kernel (v2).

Reference reformulation (verified to 3e-7 rel):
  H=8, L=2048, D=512, k_sel=4, SCALE=1/(H*L)
  qbar = sum_l queries[b,l,:]
  mc = keys[b] @ (wk @ wq^T @ qbar^T)  -> row form mc = qbar @ wqk @ keys^T
       with wqk = wq @ wk^T  (host-precomputed weight product)
  top_idx = top4(mc); w = softmax(SCALE * top_vals)
  VpT[c, l] = (values[b] @ wv_half)^T
  AwT[c, l] = sum_j w_j VpT[c, (l + idx_j) mod L]   (free-dim dynamic slices)
  out rows (reference transpose quirk): for each r in 0..3:
      out_rows(r) = Aw[r*512:(r+1)*512, :].T @ wo

Sharding: 8 cores = 4 batches x 2 channel-halves. Each core redundantly
computes the tiny front-end for its batch and produces the 1024 output
rows whose channels fall in its half.

Device program per core (instruction-count-minimized v2):
  - qbar columns via DVE tensor_reduce over host-transposed qT tiles (fp8)
  - g2col = (wqk^T @ qbar) column chunks: 16 small PE matmuls (fp16)
  - mc row [1,2048] = g2 @ keys^T on PE (rhs = fp8 kt tiles)
  - top-8 via DVE max/max_index, softmax on top-4, w broadcast via PE
  - VpT = wvh^T @ vt on PE (fp16), doubled along l for circular slices
  - AwT = sum_j w_j * dyn_slice_j(VpT) fused on DVE
    (tensor_scalar + scalar_tensor_tensor, no PE identity matmuls)
  - PE-transpose AwT -> Aw tiles, final GEMM Aw_r^T @ wo, fp32 out
Streams q/k in fp8e4m3 (selection margin verified >= 1000x device
rounding noise), v/weights fp16, intermediates fp16, accum fp32.
"""

import numpy as np

B, L, D = 4, 2048, 512
H = 8
K_SEL = 4
SCALE = 1.0 / (H * L)
N_CORES = 8
P = 128
CH = 256          # channels per core (half of 512)
DK = D // P       # 4 d-tiles


def _build_nc():
    import concourse.bass as bass
    import concourse.bacc as bacc
    import concourse.mybir as mybir
    from concourse.tile import TileContext
    from concourse.masks import make_identity

    fp32 = mybir.dt.float32
    f16 = mybir.dt.float16
    f8 = mybir.dt.float8e4
    u32 = mybir.dt.uint32
    i32 = mybir.dt.int32
    AX = mybir.AxisListType.X
    MUL = mybir.AluOpType.mult
    ADD = mybir.AluOpType.add
    Exp = mybir.ActivationFunctionType.Exp

    nc = bacc.Bacc("TRN2", target_bir_lowering=False, debug=False, num_devices=N_CORES)

    qt_dram = nc.dram_tensor("qt", [D, L], f8, kind="ExternalInput")     # queries^T
    kt_dram = nc.dram_tensor("kt", [D, L], f8, kind="ExternalInput")     # keys^T
    vt_dram = nc.dram_tensor("vt", [D, L], f16, kind="ExternalInput")    # values^T
    wqk_dram = nc.dram_tensor("wqk", [D, D], f16, kind="ExternalInput")  # wq @ wk^T
    wvh_dram = nc.dram_tensor("wvh", [D, CH], f16, kind="ExternalInput")
    wo_dram = nc.dram_tensor("wo", [D, D], f16, kind="ExternalInput")
    out_dram = nc.dram_tensor("out", [L // 2, D], fp32, kind="ExternalOutput")

    with TileContext(nc) as tc:
        with (
            tc.tile_pool(name="const", bufs=1) as cpool,
            tc.tile_pool(name="wts", bufs=1) as wts,
            tc.tile_pool(name="big", bufs=1) as big,
            tc.tile_pool(name="stream", bufs=2) as stream,
            tc.tile_pool(name="small", bufs=1) as small,
            tc.tile_pool(name="ps_fe", bufs=2, space="PSUM") as ps_fe,
            tc.tile_pool(name="ps_mm", bufs=2, space="PSUM") as ps_mm,
            tc.tile_pool(name="ps_tp", bufs=2, space="PSUM") as ps_tp,
            tc.tile_pool(name="ps_sm", bufs=1, space="PSUM") as ps_sm,
        ):
            ident = cpool.tile([P, P], fp32, tag="ident")
            make_identity(nc, ident)
            ident16 = cpool.tile([P, P], f16, tag="ident16")
            nc.scalar.copy(ident16, ident)
            ones_row = cpool.tile([1, P], f16, tag="onesr")
            nc.gpsimd.memset(ones_row, 1.0)

            # ---- DMAs: two HWDGE queues, priority-ordered ----
            qt_sb = [big.tile([P, L], f8, tag=f"qt{i}", name=f"qt{i}") for i in range(DK)]
            kt_sb = [big.tile([P, L], f8, tag=f"kt{i}", name=f"kt{i}") for i in range(DK)]
            vt_sb = [big.tile([P, L], f16, tag=f"vt{i}", name=f"vt{i}") for i in range(DK)]
            wqk_sb = [wts.tile([P, D], f16, tag=f"wqk{i}", name=f"wqk{i}") for i in range(DK)]
            wvh_sb = [wts.tile([P, CH], f16, tag=f"wvh{i}", name=f"wvh{i}") for i in range(DK)]
            wo_sb = [wts.tile([P, D], f16, tag=f"wo{i}", name=f"wo{i}") for i in range(DK)]

            # sync: qt0 qt1 kt0 kt1 vt0 vt1 ; scalar: wqk qt2 qt3 kt2 kt3 wvh vt2 vt3 wo
            for i in range(DK):
                nc.scalar.dma_start(wqk_sb[i], wqk_dram[i * P:(i + 1) * P, :])
            for i in (0, 1):
                nc.sync.dma_start(qt_sb[i], qt_dram[i * P:(i + 1) * P, :])
            for i in (2, 3):
                nc.scalar.dma_start(qt_sb[i], qt_dram[i * P:(i + 1) * P, :])
            for i in (0, 1):
                nc.sync.dma_start(kt_sb[i], kt_dram[i * P:(i + 1) * P, :])
            for i in (2, 3):
                nc.scalar.dma_start(kt_sb[i], kt_dram[i * P:(i + 1) * P, :])
            for i in range(DK):
                nc.scalar.dma_start(wvh_sb[i], wvh_dram[i * P:(i + 1) * P, :])
            for i in (0, 1):
                nc.sync.dma_start(vt_sb[i], vt_dram[i * P:(i + 1) * P, :])
            for i in (2, 3):
                nc.scalar.dma_start(vt_sb[i], vt_dram[i * P:(i + 1) * P, :])
            for i in range(DK):
                nc.sync.dma_start(wo_sb[i], wo_dram[i * P:(i + 1) * P, :])

            # ---- qbar columns via DVE free-dim reduce over qT tiles ----
            qbcol = small.tile([P, DK], fp32, tag="qbcol")
            for i in range(DK):
                nc.vector.reduce_sum(qbcol[:, i:i + 1], qt_sb[i], axis=AX)
            qb16 = small.tile([P, DK], f16, tag="qb16")
            nc.scalar.copy(qb16, qbcol)

            # ---- g2col = (wqk^T @ qbar) as column chunks [128, 4] ----
            g2c16 = small.tile([P, DK], f16, tag="g2c16")
            for m in range(DK):
                psg = ps_sm.tile([P, 1], fp32, tag="g2")
                for kk in range(DK):
                    nc.tensor.matmul(
                        psg, wqk_sb[kk][:, m * P:(m + 1) * P], qb16[:, kk:kk + 1],
                        start=(kk == 0), stop=(kk == DK - 1),
                    )
                nc.scalar.copy(g2c16[:, m:m + 1], psg)

            # ---- mc row [1, 2048] = g2 @ keys^T ----
            mc_flat = small.tile([1, L], fp32, tag="mc_flat")
            for nch in range(4):
                psm = ps_fe.tile([1, 512], fp32, tag="mc")
                for dk in range(DK):
                    nc.tensor.matmul(
                        psm, g2c16[:, dk:dk + 1],
                        kt_sb[dk][:, nch * 512:(nch + 1) * 512],
                        start=(dk == 0), stop=(dk == DK - 1),
                    )
                nc.scalar.copy(mc_flat[0:1, nch * 512:(nch + 1) * 512], psm)

            # ---- top-8 + softmax over top-4 ----
            mx8 = small.tile([1, 8], fp32, tag="mx8")
            mi8 = small.tile([1, 8], u32, tag="mi8")
            nc.vector.max(out=mx8, in_=mc_flat)
            nc.vector.max_index(out=mi8, in_max=mx8, in_values=mc_flat)
            e4 = small.tile([1, K_SEL], fp32, tag="e4")
            nc.scalar.activation(e4, mx8[0:1, 0:K_SEL], Exp, scale=float(SCALE))
            s1 = small.tile([1, 1], fp32, tag="s1")
            nc.vector.reduce_sum(s1, e4, axis=AX)
            r1 = small.tile([1, 1], fp32, tag="r1")
            nc.vector.reciprocal(r1, s1)
            w4 = small.tile([1, K_SEL], f16, tag="w4")
            nc.vector.tensor_scalar(w4, e4, r1[0:1, 0:1], None, op0=MUL)

            # broadcast w4 to [128, 4] via PE: ones_row.T @ w4
            ps_wb = ps_sm.tile([P, K_SEL], fp32, tag="wb")
            nc.tensor.matmul(ps_wb, ones_row, w4, start=True, stop=True)
            wb = small.tile([P, K_SEL], fp32, tag="wb_sb")
            nc.scalar.copy(wb, ps_wb)

            # ---- VpT = wvh^T @ vt, doubled along l ----
            vpT = big.tile([P, 2, 2 * L], f16, tag="vpT", name="vpT")
            for ct in range(2):
                for lc in range(4):
                    pv = ps_mm.tile([P, 512], fp32, tag="mm")
                    for dk in range(DK):
                        nc.tensor.matmul(
                            pv,
                            wvh_sb[dk][:, ct * P:(ct + 1) * P],
                            vt_sb[dk][:, lc * 512:(lc + 1) * 512],
                            start=(dk == 0), stop=(dk == DK - 1),
                        )
                    nc.scalar.copy(vpT[:, ct, lc * 512:(lc + 1) * 512], pv)
                nc.vector.tensor_copy(vpT[:, ct, L:2 * L], vpT[:, ct, 0:L])

            # ---- AwT = sum_j w_j * dyn_slice_j(VpT), fused on DVE ----
            # two l-regions (A: l<1024 covers r0/r1; B: rest) so PE transposes
            # of region A overlap DVE work on region B
            awT = big.tile([P, 2, L], f16, tag="awT", name="awT")
            HLF = L // 2
            s_regs = [
                nc.values_load(
                    mi8[0:1, j:j + 1].bitcast(i32),
                    engines=(mybir.EngineType.DVE,),
                    min_val=0, max_val=L - 1,
                    skip_runtime_bounds_check=True,
                ) for j in range(K_SEL)
            ]
            for reg in range(2):
                srcs = [vpT[:, :, bass.ds(s_regs[j] + reg * HLF, HLF)]
                        for j in range(K_SEL)]
                dst = awT[:, :, reg * HLF:(reg + 1) * HLF]
                nc.vector.tensor_scalar(dst, srcs[0], wb[:, 0:1], None, op0=MUL)
                for j in range(1, K_SEL):
                    nc.vector.scalar_tensor_tensor(
                        dst, srcs[j], wb[:, j:j + 1], dst, op0=MUL, op1=ADD)

            # ---- transpose AwT -> Aw tiles; final GEMM per r ----
            for r in range(4):
                aw = [small.tile([P, CH], f16, tag=f"aw_{r}_{lp}", name=f"aw{r}_{lp}")
                      for lp in range(4)]
                for ct in range(2):
                    for lp in range(4):
                        pt = ps_tp.tile([P, P], f16, tag="tp")
                        nc.tensor.transpose(
                            pt, awT[:, ct, r * 512 + lp * P: r * 512 + (lp + 1) * P],
                            ident16)
                        if (ct * 4 + lp) % 2 == 0:
                            nc.scalar.copy(aw[lp][:, ct * P:(ct + 1) * P], pt)
                        else:
                            nc.vector.tensor_copy(aw[lp][:, ct * P:(ct + 1) * P], pt)
                for cm in range(2):
                    po = ps_mm.tile([P, D], fp32, tag="mm")
                    for lp in range(4):
                        nc.tensor.matmul(
                            po, aw[lp][:, cm * P:(cm + 1) * P], wo_sb[lp],
                            start=(lp == 0), stop=(lp == DK - 1),
                        )
                    ot = stream.tile([P, D], fp32, tag="otile")
                    nc.scalar.copy(ot, po)
                    row0 = r * 256 + cm * P
                    eng = nc.sync if cm == 0 else nc.scalar
                    eng.dma_start(out_dram[row0:row0 + P, :], ot)

    nc.compile()
    return nc


_NC_CACHE = None


def _get_nc():
    global _NC_CACHE
    if _NC_CACHE is None:
        _NC_CACHE = _build_nc()
    return _NC_CACHE


def _half_cols(half):
    d0 = 32 * half
    return np.array([(cl // 32) * 64 + d0 + cl % 32 for cl in range(CH)])


def _row_index(half):
    # device row r*256 + cl  ->  full-output row i
    d0 = 32 * half
    idx = np.empty(1024, np.int64)
    for r in range(4):
        for cl in range(CH):
            i = (d0 + cl % 32) * 32 + (cl // 32) * 4 + r
            idx[r * CH + cl] = i
    return idx


def make_in_maps(queries, keys, values, wq, wk, wv, wo):
    import ml_dtypes
    f8 = ml_dtypes.float8_e4m3
    wqk = (wq.astype(np.float64) @ wk.T.astype(np.float64)).astype(np.float16)
    wo16 = wo.astype(np.float16)
    in_maps = []
    for c in range(N_CORES):
        b, half = c // 2, c % 2
        qt = np.ascontiguousarray(queries[b].T).astype(f8)
        kt = np.ascontiguousarray(keys[b].T).astype(f8)
        vt = np.ascontiguousarray(values[b].T).astype(np.float16)
        wvh = np.ascontiguousarray(wv[:, _half_cols(half)]).astype(np.float16)
        in_maps.append({
            "qt": qt, "kt": kt, "vt": vt,
            "wqk": wqk, "wvh": wvh, "wo": wo16,
        })
    return in_maps


def kernel(queries, keys, values, wq, wk, wv, wo, trace=False):
    import sys
    if "/opt/trn_rl_repo" not in sys.path:
        sys.path.insert(0, "/opt/trn_rl_repo")
    from concourse import bass_utils

    nc = _get_nc()
    in_maps = make_in_maps(queries, keys, values, wq, wk, wv, wo)
    res = bass_utils.run_bass_kernel_spmd(
        nc, in_maps, core_ids=list(range(N_CORES)), trace=trace,
    )
    out = np.empty((B, L, D), np.float32)
    for c in range(N_CORES):
        b, half = c // 2, c % 2
        out[b, _row_index(half), :] = res.results[c]["out"]
    if trace:
        return out, res
    return out
